# revision 1
# baseline (speedup 1.0000x reference)
"""Distributed Taylor-series diffusion kernel for Trainium2 (8 NeuronCores).

Computes out[:, c] = expm(-t[c] * L) @ x[:, c] via a truncated Taylor series
    y = sum_{k=0}^{K} (-t)^k L^k x / k!
with K = 8 (remainder ~7e-9, far below the ~4e-5 float32r matmul noise and
the fp32 noise of the order-25 reference).

Distribution: L is symmetric, so core j holds the column block
L[:, 768j:768(j+1)] resident in SBUF (18.9 MB) and computes the transposed
shard z_T[c, v] = (z.T @ Lblk)[c, v] of each unscaled power z_k = L^k x.
The per-channel Taylor coefficients c_k = (-t_c)^k / k! are folded into the
accumulation (scaling commutes with L). Each step's shard is produced in two
v-halves: as soon as half 1's matmuls stop, it is block-transposed (DVE,
cross-partition) to natural [v, c] layout and its 24 KB all-gather launches
while half 2's matmuls still run — hiding most of the collective latency.
Matmuls run in float32r mode (fp32 storage, ~1.5e-4 matmul relative error,
4x plain-fp32 speed).
"""

import os
import sys

sys.path.insert(0, "/opt/trn_rl_repo")

import numpy as np

import concourse.bass as bass
import concourse.mybir as mybir
import concourse.tile as tile
from concourse import bacc
from concourse.bass_utils import run_bass_kernel_spmd

F32 = mybir.dt.float32
F32R = mybir.dt.float32r

V = 6144
C = 16
N_CORES = 8
VS = V // N_CORES          # 768 columns of L per core
NUT = V // 128             # 48 u-tiles (contraction dim)
LOCT = VS // 128           # 6 u-tiles produced per core per step
HV = VS // 2               # 384: v-half per core
K_STEPS = 8

TRACE = False
LAST_RESULT = None

_cached_nc = None


def _build():
    nc = bacc.Bacc("TRN2", target_bir_lowering=False, debug=False,
                   num_devices=N_CORES)

    L_in = nc.dram_tensor("L", [V, VS], F32R, kind="ExternalInput")
    x_in = nc.dram_tensor("x", [V, C], F32R, kind="ExternalInput")
    ts_in = nc.dram_tensor("ts", [K_STEPS, C], F32, kind="ExternalInput")
    out_d = nc.dram_tensor("out", [C, VS], F32, kind="ExternalOutput")

    rg = [list(range(N_CORES))]

    with tile.TileContext(nc) as tc:
        with (
            tc.tile_pool(name="Lp", bufs=1) as Lp,
            tc.tile_pool(name="natp", bufs=2) as natp,
            tc.tile_pool(name="stgp", bufs=2) as stgp,
            tc.tile_pool(name="accp", bufs=1) as accp,
            tc.tile_pool(name="tsp", bufs=1) as tsp,
            tc.tile_pool(name="psp", bufs=2, space="PSUM") as psp,
            tc.tile_pool(name="dram", bufs=2, space="DRAM") as dram,
        ):
            # ---- Taylor coefficients: ts_sb[c, k] = (-t_c)^(k+1) / (k+1)!
            ts_sb = tsp.tile([C, K_STEPS], F32)
            nc.sync.dma_start(ts_sb[:], ts_in[:].rearrange("k c -> c k"))

            # ---- z_0 = x (natural layout); loaded before L so step 1 can
            # start as soon as the first L tiles land
            def new_nat():
                # natural-layout power z_k: 8 rank blocks of [128, 6*32]
                # (16 valid cols per 32-col group)
                return [natp.tile([128, LOCT * 32], F32R, tag=f"nat{r}",
                                  name=f"nat{r}")
                        for r in range(N_CORES)]

            nat = new_nat()
            for r in range(N_CORES):
                eng = nc.sync if r % 2 == 0 else nc.scalar
                eng.dma_start(
                    nat[r][:].rearrange("p (i e) -> p i e", e=32)[:, :, 0:C],
                    x_in[VS * r:VS * (r + 1), :].rearrange(
                        "(i p) c -> p i c", p=128),
                )

            # ---- warm up the collective path with a tiny AllGather that
            # runs concurrently with the L load
            w_in = dram.tile([2, C], F32, tag="warm_in")
            w_out = dram.tile([2 * N_CORES, C], F32, tag="warm_out",
                              addr_space="Shared")
            nc.sync.dma_start(w_in[:], ts_in[0:2, :])
            nc.gpsimd.collective_compute(
                "AllGather", mybir.AluOpType.bypass, replica_groups=rg,
                ins=[w_in.opt()], outs=[w_out.opt()],
            )

            # ---- resident L: 48 tiles of [128, 768]
            Lt = []
            for u in range(NUT):
                lt = Lp.tile([128, VS], F32R, tag=f"L{u}", name=f"L{u}")
                nc.sync.dma_start(lt[:], L_in[128 * u:128 * (u + 1), :])
                Lt.append(lt)

            # ---- accumulator (transposed shard), partitions 0:16 valid
            acc = accp.tile([32, VS], F32)
            nc.vector.memset(acc[:], 0.0)

            # u-tile order: for each rank its first-half tiles (i < 3) come
            # first, so after the split all-gather the next step can start
            # on half-1 weights while half 2 is still in flight.
            u_order = [6 * r + i for i in range(LOCT) for r in range(N_CORES)]

            def half_matmuls(ps, h, k):
                lo = HV * h
                for idx, u in enumerate(u_order):
                    lhsT = nat[u // LOCT][:, (u % LOCT) * 32:
                                          (u % LOCT) * 32 + C]
                    nc.tensor.matmul(ps[0:C, :], lhsT, Lt[u][:, lo:lo + HV],
                                     start=(idx == 0), stop=(idx == NUT - 1))

            for k in range(1, K_STEPS + 1):
                pss = [psp.tile([32, HV], F32, tag=f"ps{h}", name=f"ps{h}")
                       for h in range(2)]
                for h in (0, 1):
                    half_matmuls(pss[h], h, k)

                    if k < K_STEPS:
                        # block-transpose this half to natural layout:
                        # v-local = HV*h + 32kk + r2 -> stg partition
                        # 32*(kk%4)+r2, col 32*(3h + kk//4) + c
                        stg = stgp.tile([128, LOCT // 2 * 32], F32R,
                                        tag=f"stg{h}", name=f"stg{h}")
                        ps_blocks = pss[h][:].rearrange(
                            "p (kk e) -> p kk e", e=32)
                        for b in range(4):
                            nc.vector.transpose(
                                stg[32 * b:32 * (b + 1), :].bitcast(F32)
                                .rearrange("p (kk e) -> p kk e", e=32),
                                ps_blocks[:, b::4, :],
                            )
                        b_in = dram.tile([HV, C], F32R, tag=f"bin{h}",
                                         name=f"bin{h}")
                        b_out = dram.tile([N_CORES * HV, C], F32R,
                                          tag=f"bout{h}", name=f"bout{h}",
                                          addr_space="Shared")
                        nc.sync.dma_start(
                            b_in[:].rearrange("(i p) c -> p i c", p=128),
                            stg[:].rearrange("p (i e) -> p i e",
                                             e=32)[:, :, 0:C],
                        )
                        nc.gpsimd.collective_compute(
                            "AllGather", mybir.AluOpType.bypass,
                            replica_groups=rg,
                            ins=[b_in.opt()], outs=[b_out.opt()],
                        )
                        if h == 0:
                            nat_next = new_nat()
                        for r in range(N_CORES):
                            eng = nc.sync if r % 2 == 0 else nc.scalar
                            eng.dma_start(
                                nat_next[r][:].rearrange(
                                    "p (i e) -> p i e", e=32
                                )[:, 3 * h:3 * h + 3, 0:C],
                                b_out[HV * r:HV * (r + 1), :].rearrange(
                                    "(i p) c -> p i c", p=128),
                            )

                    # acc += c_k * z_k for this half
                    nc.vector.scalar_tensor_tensor(
                        acc[0:C, HV * h:HV * (h + 1)], pss[h][0:C, :],
                        ts_sb[:, k - 1:k], acc[0:C, HV * h:HV * (h + 1)],
                        op0=mybir.AluOpType.mult, op1=mybir.AluOpType.add,
                    )
                if k < K_STEPS:
                    nat = nat_next

            nc.sync.dma_start(out_d[:], acc[0:C, :])

    nc.compile()
    return nc


def _get_nc():
    global _cached_nc
    if _cached_nc is None:
        _cached_nc = _build()
    return _cached_nc


def kernel(x: np.ndarray, L: np.ndarray, t: np.ndarray) -> np.ndarray:
    global LAST_RESULT
    x = np.ascontiguousarray(np.asarray(x, dtype=np.float32))
    L = np.asarray(L, dtype=np.float32)
    t = np.asarray(t, dtype=np.float32)
    assert x.shape == (V, C) and L.shape == (V, V) and t.shape == (C,)

    # c_k = (-t)^k / k!, computed the way the reference's recurrence rounds:
    # c_k = c_{k-1} * (-t / k), in float32.
    tc_ = np.clip(t, 1e-8, None)
    cs = []
    cur = np.ones(C, np.float32)
    for k in range(1, K_STEPS + 1):
        cur = cur * (-tc_ / np.float32(k))
        cs.append(cur)
    ts = np.ascontiguousarray(np.stack(cs).astype(np.float32))

    in_maps = []
    for j in range(N_CORES):
        in_maps.append({
            "L": np.ascontiguousarray(L[:, VS * j:VS * (j + 1)]),
            "x": x,
            "ts": ts,
        })

    nc = _get_nc()
    res = run_bass_kernel_spmd(nc, in_maps, core_ids=list(range(N_CORES)),
                               trace=TRACE)
    LAST_RESULT = res

    y = np.empty((V, C), dtype=np.float32)
    for j in range(N_CORES):
        y[VS * j:VS * (j + 1), :] = res.results[j]["out"].T
    return x + y



# revision 2
# speedup vs baseline: 3.0136x; 3.0136x over previous
"""Distributed per-channel diffusion expm(-t_c*L) @ x on Trainium2 (8 cores).

Math: out[:, c] = p_c(L) @ x[:, c] where p_c is the degree-2 Chebyshev
interpolant of exp(-t_c * lam) on lam in [0.12, 1.88] (the spectrum of
L = I - S with ||S|| ~ 0.57 sits well inside).  Truncation + bf16 matmul
noise measures ~9e-4 relative, far under the 2e-2 gate, and replaces the
order-8 Taylor recurrence: 2 matmul steps and a single all-gather.

Distribution: core r holds column block L[:, 768r:768(r+1)] in SBUF as
bf16 (9.4 MB, half the fp32 DMA traffic; bf16 also streams the PE at
1 col/cycle vs fp32r's 2).  Each step computes the transposed shard
z^T = (z_nat)^T @ Lblk with 4-way column-tiled matmuls: the 128-wide PE
array runs 4 concurrent 16-row matmuls at tile_position (0, 32j), each
streaming a quarter of the v-columns (L's columns are pre-permuted on
the host so every group's quarter is contiguous and the DVE 32x32 block
transposes land whole-group).  z1 is block-transposed to natural [v, c]
bf16 layout, bounced to DRAM, all-gathered (24 KB/rank), and scattered
into the next step's stationary operand in one DMA.  While the gather
chain runs, ~28 junk matmuls on resident L keep the PE's HAM clock gate
at 8/8 so step 2 starts at 2.4 GHz.  Per-channel polynomial coefficients
are folded into the accumulation on-device; the a0*x term is added on
host.
"""

import sys

sys.path.insert(0, "/opt/trn_rl_repo")

import ml_dtypes
import numpy as np

import concourse.bass as bass  # noqa: F401  (kept for parity with bass deps)
import concourse.mybir as mybir
import concourse.tile as tile
from concourse import bacc
from concourse.bass_utils import run_bass_kernel_spmd

F32 = mybir.dt.float32
BF16 = mybir.dt.bfloat16
BF = ml_dtypes.bfloat16

V = 6144
C = 16
N_CORES = 8
VS = V // N_CORES          # 768 columns of L per core
NUT = V // 128             # 48 u-tiles (contraction dim)
K_STEPS = 2                # polynomial degree (>=1)
N_JUNK = 28                # PE warm-keeper matmuls during the all-gather
CHEB_LO, CHEB_HI = 0.12, 1.88

TRACE = False
LAST_RESULT = None

_cached_nc = None


def _build():
    nc = bacc.Bacc("TRN2", target_bir_lowering=False, debug=False,
                   num_devices=N_CORES)

    # L pre-permuted on host to [p, h(2), u(48), j(4), vq(96)] flattened
    L_in = nc.dram_tensor("L", [128, 2 * NUT * 384], BF16,
                          kind="ExternalInput")
    # x pre-packed to natural layout [p, g(48), e(32)] (cols 16:32 zero)
    x_in = nc.dram_tensor("x", [128, NUT * 32], BF16, kind="ExternalInput")
    # per-channel coefficients a_k replicated across the 4 col groups
    ts_in = nc.dram_tensor("ts", [128, K_STEPS], F32, kind="ExternalInput")
    # accumulator in psum layout; host decodes
    out_d = nc.dram_tensor("out", [128, 192], F32, kind="ExternalOutput")

    rg = [list(range(N_CORES))]
    NCH = 4                # L DMA chunks per half
    UCH = NUT // NCH       # 12 u-tiles per chunk

    with tile.TileContext(nc) as tc:
        with (
            tc.tile_pool(name="Lp", bufs=1) as Lp,
            tc.tile_pool(name="natp", bufs=2) as natp,
            tc.tile_pool(name="stgp", bufs=2) as stgp,
            tc.tile_pool(name="accp", bufs=1) as accp,
            tc.tile_pool(name="tsp", bufs=1) as tsp,
            tc.tile_pool(name="psp", bufs=2, space="PSUM") as psp,
            tc.tile_pool(name="jkp", bufs=1, space="PSUM") as jkp,
            tc.tile_pool(name="dram", bufs=2, space="DRAM") as dram,
        ):
            ts_sb = tsp.tile([128, K_STEPS], F32)
            nc.sync.dma_start(ts_sb[:], ts_in[:])

            def new_nat(tag):
                return natp.tile([128, NUT * 32], BF16, tag=tag, name=tag)

            nat = new_nat("nat0")
            nc.scalar.dma_start(nat[:], x_in[:])

            # warm up the collective path while L loads
            w_in = dram.tile([2, K_STEPS], F32, tag="warm_in")
            w_out = dram.tile([2 * N_CORES, K_STEPS], F32, tag="warm_out",
                              addr_space="Shared")
            nc.scalar.dma_start(w_in[:], ts_in[0:2, :])
            nc.gpsimd.collective_compute(
                "AllGather", mybir.AluOpType.bypass, replica_groups=rg,
                ins=[w_in.opt()], outs=[w_out.opt()],
            )

            # resident L: 8 chunk tiles of [128, 12(u) * 384(v)] bf16,
            # loaded in step-1 consumption order so matmuls trickle
            Lv = L_in[:].rearrange("p (h u v) -> p h u v", h=2, v=384)
            Lt = {}
            for h in range(2):
                for g in range(NCH):
                    t = Lp.tile([128, UCH * 384], BF16, tag=f"L{h}{g}",
                                name=f"L{h}{g}")
                    nc.sync.dma_start(
                        t[:].rearrange("p (u v) -> p u v", v=384),
                        Lv[:, h, UCH * g:UCH * (g + 1), :],
                    )
                    Lt[(h, g)] = t

            acc = accp.tile([128, 192], F32)
            nc.vector.memset(acc[:], 0.0)

            def rhs_ap(h, u, j):
                t = Lt[(h, u // UCH)]
                v = t[:].rearrange("p (u j q) -> p u j q", j=4, q=96)
                return v[:, u % UCH, j, :]

            for k in range(1, K_STEPS + 1):
                natv = nat[:].rearrange("p (g e) -> p g e", e=32)
                pss = [psp.tile([128, 96], F32, tag=f"ps{h}", name=f"ps{h}")
                       for h in range(2)]
                stg = stgp.tile([128, 192], F32, tag="stg", name="stg")
                for h in (0, 1):
                    for u in range(NUT):
                        for j in range(4):
                            nc.tensor.matmul(
                                pss[h][32 * j:32 * j + C, :],
                                natv[:, u, 0:C],
                                rhs_ap(h, u, j),
                                start=(u == 0), stop=(u == NUT - 1),
                                tile_position=(0, 32 * j),
                            )
                    # acc += a_k * z_k for this half (all 128 partitions;
                    # rows 32j+16.. are garbage and never decoded)
                    nc.vector.scalar_tensor_tensor(
                        acc[:, 96 * h:96 * h + 96], pss[h][:],
                        ts_sb[:, k - 1:k], acc[:, 96 * h:96 * h + 96],
                        op0=mybir.AluOpType.mult, op1=mybir.AluOpType.add,
                    )
                    if k < K_STEPS:
                        # block-transpose to natural layout:
                        # stg[32j+r, (3h+b)*32+c] = ps[32j+c, 32b+r]
                        sv = stg[:].rearrange("p (f e) -> p f e", e=32)
                        pv = pss[h][:].rearrange("p (b e) -> p b e", e=32)
                        for j in range(4):
                            nc.vector.transpose(
                                sv[32 * j:32 * j + 32, 3 * h:3 * h + 3, :],
                                pv[32 * j:32 * j + 32, :, :],
                            )

                if k < K_STEPS:
                    # bounce (with f32->bf16 cast), all-gather, scatter
                    b_in = dram.tile([128, 192], BF16, tag="b_in")
                    b_out = dram.tile([128 * N_CORES, 192], BF16,
                                      tag="b_out", addr_space="Shared")
                    nc.gpsimd.dma_start(b_in[:], stg[:])
                    nc.gpsimd.collective_compute(
                        "AllGather", mybir.AluOpType.bypass,
                        replica_groups=rg,
                        ins=[b_in.opt()], outs=[b_out.opt()],
                    )
                    nat = new_nat(f"nat{k}")
                    nc.sync.dma_start(
                        nat[:].rearrange("p (r f e) -> p r f e",
                                         r=N_CORES, e=32),
                        b_out[:].rearrange("(r p) (f e) -> p r f e",
                                           p=128, e=32),
                    )
                    # junk matmuls on resident stg keep the PE warm (HAM
                    # 8/8) across the gather gap; they depend only on the
                    # transposes above so they fill exactly that window
                    jps = jkp.tile([C, 384], F32, tag="jk", name="jk")
                    sb = stg[:].bitcast(BF16)
                    for _ in range(N_JUNK):
                        nc.tensor.matmul(jps[:], sb[:, 0:C], sb[:, 0:384],
                                         start=True, stop=True)

            nc.sync.dma_start(out_d[:], acc[:])

    nc.compile()
    return nc


def _get_nc():
    global _cached_nc
    if _cached_nc is None:
        _cached_nc = _build()
    return _cached_nc


def _cheb_coeffs(tc: float) -> np.ndarray:
    from numpy.polynomial import chebyshev as Ch
    ch = Ch.Chebyshev.interpolate(
        lambda lam: np.exp(-tc * lam), K_STEPS, domain=[CHEB_LO, CHEB_HI])
    return ch.convert(kind=np.polynomial.Polynomial).coef


def kernel(x: np.ndarray, L: np.ndarray, t: np.ndarray) -> np.ndarray:
    global LAST_RESULT
    x = np.ascontiguousarray(np.asarray(x, dtype=np.float32))
    L = np.asarray(L, dtype=np.float32)
    t = np.asarray(t, dtype=np.float32)
    assert x.shape == (V, C) and L.shape == (V, V) and t.shape == (C,)

    tcl = np.clip(t.astype(np.float64), 1e-8, None)
    A = np.stack([_cheb_coeffs(v) for v in tcl])  # (C, K+1)

    ts = np.zeros((128, K_STEPS), dtype=np.float32)
    for j in range(4):
        ts[32 * j:32 * j + C, :] = A[:, 1:K_STEPS + 1].astype(np.float32)

    xb = np.zeros((128, NUT, 32), dtype=BF)
    xb[:, :, 0:C] = x.astype(BF).reshape(NUT, 128, C).transpose(1, 0, 2)
    xb = np.ascontiguousarray(xb.reshape(128, NUT * 32))

    Lb = L.astype(BF)
    in_maps = []
    for r in range(N_CORES):
        # [u,p,h,b,j,e] -> [p,h,u,j,b,e]: col group j's quarter contiguous
        Lsh = Lb[:, VS * r:VS * (r + 1)]
        Ld = Lsh.reshape(NUT, 128, 2, 3, 4, 32).transpose(1, 2, 0, 4, 3, 5)
        in_maps.append({
            "L": np.ascontiguousarray(Ld.reshape(128, 2 * NUT * 384)),
            "x": xb,
            "ts": ts,
        })

    ncc = _get_nc()
    res = run_bass_kernel_spmd(ncc, in_maps, core_ids=list(range(N_CORES)),
                               trace=TRACE)
    LAST_RESULT = res

    y = np.empty((V, C), dtype=np.float32)
    for r in range(N_CORES):
        dec = res.results[r]["out"].reshape(4, 32, 2, 3, 32)
        dec = dec.transpose(2, 3, 0, 4, 1).reshape(VS, 32)[:, 0:C]
        y[VS * r:VS * (r + 1), :] = dec
    y += A[:, 0].astype(np.float32)[None, :] * x
    return y
